# revision 46
# baseline (speedup 1.0000x reference)
"""Trainium2 Bass kernel for nn_MultiInfAffine.

Math (reference):
    mu_n = mus / ||mus||_D                          [L=6, D=16, K=64]
    t    = <x, mu_n>                                 per (l, n, k)
    cost = 0.5 * arccos(t)^2 + alpha
    mc_l = 0.1 * ln sum_k exp(-cost/0.1)
    F    = recurrence over l:  F = wv_l relu(F) + (1-wv_l) mc_l,  wv = exp(-ws^2)
    out  = 0.1 * ln(1 + exp(-10 F))

Device chain per element (1 custom-DVE pass + 1 ACT pass):
    v    = t - RHO            [PE matmul, fp16, contract 17 via ones-row]
    q    = v (v^2 + QA v + QB)(v^2 + QD)            [DVE custom MIA_Q5B_ANT]
    E    = DErf(KAP q + BET) = 2/sqrt(pi) exp(-(KAP q + BET)^2)  -> bf16 [ACT]
    S_l  = sum_k w_k E_k                            [PE reduce matmul]
  where M(t) = KAP (t-RHO)((t-RHO)^2+QA(t-RHO)+QB)((t-RHO)^2+QD) + BET is a
  quintic with M(t)^2 ~= 5*arccos(t)^2 + CEXP (weighted fit; exponent err
  <= 7.3e-4 wherever a term can be within e^-25 of its point's dominant
  term), and w_k = sqrt(pi)/2 * exp(CEXP - 10 alpha_k) makes
  w_k E_k ~= exp(-10 cost). Then a small tail (Ln + 6-step recurrence +
  smooth-min) on re-tiled data.

Layout: 128 SBUF partitions = 2 layers x 64 components ("plane" g covers
layers 2g, 2g+1; 3 planes). Points stream along the free axis in subtiles of
512 (one PSUM bank per plane). ACT runs only DErf + Copy in the main loop
(both in the erf_derivative table set; no table swaps).

HW-measured engine choices (microbenched on this trn2):
  - main matmul fp16 0.86 ns/col vs fp32r 1.41 (bf16 pathological, 7.7)
  - custom DVE, single-uop all-immediate spec: 0.76 ns/col from SBUF,
    1.47 from PSUM (intrinsic PSUM-read penalty; still cheaper than an
    ACT copy at ~1.9/col would be)
  - ACT (N+352)/1.2GHz per instr: paired subtiles per DErf instruction
  - reduce matmul bf16 contract-128: 0.23 ns/col
  - S copied out of PSUM in [24,512] groups of 4 subtiles (zero-padded
    stationary columns place group-slot j at rows 4l+j, base partition 0)
"""

import numpy as np
import ml_dtypes

import concourse.bacc as bacc
import concourse.tile as tile
from concourse import mybir
from concourse.bass_utils import run_bass_kernel_spmd

N, D, L, K = 250000, 16, 6, 64
NCORES = 8
NPC = N // NCORES  # 31250 true points per core

# tiling (per core)
SC = 512       # points per subtile (columns; one PSUM bank per plane)
SVG = 4        # subtiles per S-group (one [24, SC] PSUM bank, one copy)
NSUB = 64      # subtiles per core (16 S-groups)
NPAD = SC * NSUB  # 32768 padded points per core
T = NPAD // 128   # 256 point-columns in the tail layout

# M(t) = KAP*v*(v^2+QA*v+QB)*(v^2+QD) + BET, v = t-RHO; M^2 ~= 5 arccos^2 + CEXP
CEXP = 8.0
RHO = 1.650216200888483
QA = 6.037997075652353
QB = 12.799226390484804
QD = 4.497047893926167
KAP = 0.046074945478925385
BET = -1.4581460932891892

F32 = mybir.dt.float32
F32R = mybir.dt.float32r
BF16 = mybir.dt.bfloat16
FP16 = mybir.dt.float16
AF = mybir.ActivationFunctionType
ALU = mybir.AluOpType

# ---- custom DVE op registration (idempotent, at import) ------------------- #

def _register_ops():
    import concourse.dve_ops as dve_ops
    from concourse.dve_spec import Spec, Src0, C0, C1, C2, sq, lower, _has_src1
    from concourse.dve_uop import DveOpSpec

    def mk(name, body, reference):
        if name in dve_ops._SUB_OPCODE_FOR_NAME:
            return next(op for op in dve_ops.OPS if op.name == name)
        spec = Spec(body=body, reference=reference)
        row = dve_ops._CUSTOM_DVE_ROW_BASE + len(dve_ops.OPS)
        assert row < 0x20
        shas = {}
        for ver in ("v3", "v4"):
            shas[ver] = DveOpSpec(
                name=name, opcode=row, uops=lower(spec, ver=ver),
                rd1_en=_has_src1(spec),
            ).sha(ver)
        op = dve_ops.DveOp(name, spec, False, shas)
        dve_ops.OPS.append(op)
        dve_ops._SUB_OPCODE_FOR_NAME[name] = row
        dve_ops.CUSTOM_DVE_SPECS[name] = spec
        return op

    # out = v(v^2 + C0 v + C1)(v^2 + C2) with v = in0. All-immediate
    # single-uop spec: a C3/Src1-latch variant measured 2.2x slower.
    s = sq(Src0)
    q5_body = ((s + (Src0 * C0 + C1)) * Src0) * (s + C2)

    def q5_ref(in0, in1, c0, c1, c2):
        ss = in0 * in0
        return ((ss + in0 * c0 + c1) * in0) * (ss + c2)

    return mk("MIA_Q5B_ANT", q5_body, q5_ref)


MIA_Q5 = _register_ops()


def _build(nsub=NSUB, sc=SC, wv=None, repeat=1):
    """Build the per-core Bass program. wv: np.float32[L] = exp(-ws^2).
    repeat > 1 wraps the whole body in a HW loop (for timing; idempotent)."""
    assert wv is not None
    npad = nsub * sc

    nc = bacc.Bacc()

    xst = nc.dram_tensor("xst", [D + 1, npad], FP16, kind="ExternalInput")
    mu = nc.dram_tensor("mu", [D + 1, 3, 128], FP16, kind="ExternalInput")
    # ow[:, g, j, :]: reduce weights for group-slot j, padded with 6j leading
    # zero columns so the matmul lands rows [6j, 6j+6) of the group tile
    # while accumulating zeros into rows [0, 6j)
    ow = nc.dram_tensor("ow", [128, 3, SVG, 6 * SVG], BF16,
                        kind="ExternalInput")
    fout = nc.dram_tensor("fout", [npad], F32, kind="ExternalOutput")
    # S staging, group-packed, (h G t)-ordered columns: row 4l+j, col
    # h*(ngrp*T) + G*T + t = S_l(point (SVG*G+j)*SC + h*T + t)
    sd = nc.dram_tensor("sd", [6 * SVG, npad // SVG], F32)

    # recurrence constants
    A = [float(wv[l]) for l in range(L)]
    B = [float((1.0 - wv[l]) * 0.1) for l in range(L)]

    with tile.TileContext(nc) as tc:
        with (
            tc.tile_pool(name="singles", bufs=1) as singles,
            tc.tile_pool(name="vpsum", bufs=2, space="PSUM") as vpool,
            tc.tile_pool(name="spsum", bufs=2, space="PSUM") as spool,
            tc.tile_pool(name="q5", bufs=3) as q5pool,
            tc.tile_pool(name="e", bufs=3) as epool,
            tc.tile_pool(name="sv", bufs=3) as svpool,
            tc.tile_pool(name="tail", bufs=1) as tailpool,
        ):
            mu_sb = singles.tile([D + 1, 3, 128], FP16)
            nc.sync.dma_start(out=mu_sb[:], in_=mu[:])
            ow_sb = singles.tile([128, 3, SVG, 6 * SVG], BF16)
            nc.sync.dma_start(out=ow_sb[:], in_=ow[:])
            bet_sb = singles.tile([128, 1], F32)
            nc.vector.memset(bet_sb[:], float(BET))
            # whole-core input: one DMA, 17 descriptors (vs one per subtile)
            xs_all = singles.tile([D + 1, nsub * sc], FP16)
            nc.sync.dma_start(out=xs_all[:], in_=xst[:])

            args = (nc, tc, nsub, sc, A, B, xs_all, fout, sd,
                    mu_sb, ow_sb, bet_sb,
                    vpool, spool, q5pool, epool, svpool, tailpool)
            if repeat > 1:
                with tc.For_i(0, repeat, 1):
                    _emit_body(*args)
            else:
                _emit_body(*args)

    nc.compile()
    return nc


def _emit_body(nc, tc, nsub, sc, A, B, xs_all, fout, sd,
               mu_sb, ow_sb, bet_sb,
               vpool, spool, q5pool, epool, svpool, tailpool):
    npad = nsub * sc
    t_cols = npad // 128
    assert nsub % SVG == 0 and sc == 2 * t_cols

    mc = tailpool.tile([128, 6, t_cols], F32)
    ngrp = nsub // SVG
    nreal = nsub - 2  # trailing pad-only subtiles: skip their work entirely
    assert nreal % 2 == 0

    s_grp = q5_t = None
    for sidx in range(nreal):
        c0 = sidx * sc
        v_t = vpool.tile([128, 3, sc], F32, tag="v")
        for g in range(3):
            nc.tensor.matmul(v_t[:, g, :], mu_sb[:, g, :],
                             xs_all[:, c0:c0 + sc])
        pj = sidx % 2
        if pj == 0:
            q5_t = q5pool.tile([128, 6, sc], F32, tag="q5")
        nc.vector._custom_dve(
            MIA_Q5, out=q5_t[:, 3 * pj:3 * pj + 3, :], in0=v_t[:],
            s0=QA, s1=QB, imm2=QD,
        )
        if pj == 0:
            continue
        # one big ACT per pair of subtiles
        e_t = epool.tile([128, 6, sc], BF16, tag="e")
        nc.scalar.activation(e_t[:], q5_t[:], AF.Derivative_Erf,
                             scale=KAP, bias=bet_sb[:])
        for pj2 in range(2):
            s2 = sidx - 1 + pj2
            j = s2 % SVG
            if j == 0:
                s_grp = spool.tile([6 * SVG, sc], F32, tag="s")
            last = j == SVG - 1 or s2 == nreal - 1
            for g in range(3):
                # slot j writes rows 4l+j (l = layer); others accumulate 0
                nc.tensor.matmul(s_grp[:], ow_sb[:, g, j, :],
                                 e_t[:, 3 * pj2 + g, :],
                                 start=(j == 0 and g == 0),
                                 stop=(last and g == 2),
                                 skip_group_check=True)
            if last:
                G = s2 // SVG
                sv_t = svpool.tile([6 * SVG, sc], F32, tag="sv")
                nc.scalar.activation(sv_t[:], s_grp[:], AF.Copy)
                sd_w = sd[:].rearrange("r (h G t) -> r G h t",
                                       h=2, G=ngrp, t=t_cols)
                nc.sync.dma_start(
                    out=sd_w[:, G, :, :],
                    in_=sv_t[:].rearrange("r (h t) -> r h t", h=2))
                # point (4G+j')*512 + 256h + t -> mc[8G+2j'+h, l, t]; the
                # l partition->free crossing must hop through DRAM (sd)
                sd_r = sd[:].rearrange("(l j) (h G t) -> l G (j h) t",
                                       l=6, h=2, G=ngrp, t=t_cols)
                for l in range(L):
                    nc.sync.dma_start(
                        out=mc[8 * G:8 * G + 8, l, :],
                        in_=sd_r[l, G, :, :],
                    )

    # ---- tail: Ln, recurrence, smooth-min, store
    nc.scalar.activation(mc[:], mc[:], AF.Ln)
    for l in range(L):
        nc.vector.tensor_scalar_mul(mc[:, l, :], mc[:, l, :], B[l])
    f_t = tailpool.tile([128, t_cols], F32)
    nc.vector.tensor_copy(f_t[:], mc[:, 0, :])
    for l in range(1, L):
        nc.vector.tensor_scalar_max(f_t[:], f_t[:], 0.0)
        nc.vector.scalar_tensor_tensor(
            out=f_t[:], in0=f_t[:], scalar=A[l], in1=mc[:, l, :],
            op0=ALU.mult, op1=ALU.add,
        )
    nc.scalar.activation(f_t[:], f_t[:], AF.Exp, scale=-10.0)
    nc.scalar.activation(f_t[:], f_t[:], AF.Ln, bias=1.0)
    nc.vector.tensor_scalar_mul(f_t[:], f_t[:], 0.1)
    nc.sync.dma_start(
        out=fout[:].rearrange("(p t) -> p t", p=128), in_=f_t[:]
    )


def _host_prep(xs, mus, alphas, ws, npad_per_core=NPAD, ncores=NCORES):
    """Returns (shared inputs dict, list of per-core xst arrays, wv)."""
    mus = np.asarray(mus, np.float32)
    alphas = np.asarray(alphas, np.float32)
    ws = np.asarray(ws, np.float32)
    xs = np.asarray(xs, np.float32)

    mu_n = mus / np.linalg.norm(mus, axis=1, keepdims=True)  # [L, D, K]
    # mu layout: [17, 3, 128]; column j of plane g is (layer 2g + j//64, k = j%64)
    mu_aug = np.zeros((D + 1, 3, 128), np.float32)
    for g in range(3):
        for half in range(2):
            layer = 2 * g + half
            mu_aug[:D, g, 64 * half:64 * half + 64] = mu_n[layer]
    mu_aug[D, :, :] = -RHO  # ones-row coefficient: v = t - RHO

    # reduction weights: sqrt(pi)/2 * exp(CEXP - 10 alpha); group-slot j
    # places layer l at output row 4l+j, other columns zero (accumulate 0)
    ow = np.zeros((128, 3, SVG, 6 * SVG), np.float32)
    for g in range(3):
        for half in range(2):
            layer = 2 * g + half
            w = ((np.sqrt(np.pi) / 2.0)
                 * np.exp(CEXP - 10.0 * alphas[layer].astype(np.float64))
                 ).astype(np.float32)
            for j in range(SVG):
                ow[64 * half:64 * half + 64, g, j, 4 * layer + j] = w
    ow = ow.astype(ml_dtypes.bfloat16)

    wv = np.exp(-ws.astype(np.float32) ** 2).astype(np.float32)

    n = xs.shape[0]
    per = n // ncores
    xst_list = []
    for c in range(ncores):
        shard = xs[c * per:(c + 1) * per]
        aug = np.ones((shard.shape[0], D + 1), np.float32)
        aug[:, :D] = shard
        pad = np.zeros((npad_per_core, D + 1), np.float32)
        pad[:, D] = 1.0  # pad points: x = 0 -> v = -RHO, harmless
        pad[:shard.shape[0]] = aug
        xst_list.append(
            np.ascontiguousarray(pad.T).astype(np.float16))  # [17, npad]
    return {"mu": mu_aug.astype(np.float16), "ow": ow}, xst_list, wv


def prepare(xs, mus, alphas, ws, repeat=1):
    """Build the Bass program and per-core input maps."""
    shared, xst_list, wv = _host_prep(xs, mus, alphas, ws)
    nc = _build(wv=wv, repeat=repeat)
    in_maps = [dict(shared, xst=xst_list[c]) for c in range(NCORES)]
    return nc, in_maps


def kernel(xs, mus, alphas, ws, trace=False, tmpdir=None):
    nc, in_maps = prepare(xs, mus, alphas, ws)
    res = run_bass_kernel_spmd(
        nc, in_maps, core_ids=list(range(NCORES)), trace=trace, tmpdir=tmpdir
    )
    per = N // NCORES
    out = np.concatenate([res.results[c]["fout"][:per] for c in range(NCORES)])
    kernel.last_results = res
    return out.astype(np.float32)
